# revision 21
# baseline (speedup 1.0000x reference)
import numpy as np
import ml_dtypes

import concourse.bass as bass
import concourse.bacc as bacc
import concourse.tile as tile
from concourse import mybir
from concourse.bass_utils import run_bass_kernel_spmd

B, T, F, U, NCLS = 512, 512, 128, 64, 10
NCORES = 8
BC = B // NCORES          # 64 batch rows per core
WS = 8                    # timesteps per PSUM window
NW = T // WS              # 64 windows
TCH = 64                  # timesteps per DMA chunk
NCHUNK = T // TCH

f32 = mybir.dt.float32
bf16 = mybir.dt.bfloat16
AF = mybir.ActivationFunctionType
OP = mybir.AluOpType
BF = ml_dtypes.bfloat16

TRACE = False
LAST_RESULTS = None


def build_nc(nzrec: bool, nzb0h: bool, nzzr: bool) -> bass.Bass:
    nc = bacc.Bacc(None, target_bir_lowering=False)

    # x pre-transposed on host: [F, T, BC] bf16.  z-columns of Wzr/Uzr are
    # host-negated so the sigmoid needs no scale vector: S[0:U] = sigmoid(-az)
    # = 1-z, S[U:2U] = sigmoid(ar) = r.
    xT = nc.dram_tensor("xT", [F, T, BC], bf16, kind="ExternalInput")
    Wzr = nc.dram_tensor("Wzr", [F, 2 * U], bf16, kind="ExternalInput")
    Wh = nc.dram_tensor("Wh", [F, U], bf16, kind="ExternalInput")
    Uzr = nc.dram_tensor("Uzr", [U, 2 * U], bf16, kind="ExternalInput")
    Uh = nc.dram_tensor("Uh", [U, U], bf16, kind="ExternalInput")
    bzr = nc.dram_tensor("bzr", [2 * U, 1], f32, kind="ExternalInput")
    b1h = nc.dram_tensor("b1h", [U, 1], f32, kind="ExternalInput")
    b0h = nc.dram_tensor("b0h", [U, 1], f32, kind="ExternalInput")
    hout = nc.dram_tensor("hout", [U, BC], bf16, kind="ExternalOutput")

    with tile.TileContext(nc) as tc:
        with (
            tc.tile_pool(name="const", bufs=1) as cpool,
            tc.tile_pool(name="xchunk", bufs=2) as xpool,
            tc.tile_pool(name="hbuf", bufs=1) as hpool,
            tc.tile_pool(name="spool", bufs=3) as spool,
            tc.tile_pool(name="dpool", bufs=3) as dpool,
            tc.tile_pool(name="ddpool", bufs=2) as ddpool,
            tc.tile_pool(name="xhpool", bufs=3) as xhpool,
        ):
            # ---- start the big x-chunk DMAs first (overlap const setup) ----
            xs_tiles = {}

            def emit_dma(c):
                xsb = xpool.tile([F, TCH, BC], bf16, name="xsb")
                nc.sync.dma_start(xsb, xT[:, c * TCH:(c + 1) * TCH, :])
                xs_tiles[c] = xsb

            emit_dma(0)
            emit_dma(1)

            # ---- constants to SBUF ----
            wzr_sb = cpool.tile([F, 2 * U], bf16)
            nc.sync.dma_start(wzr_sb, Wzr[:, :])
            wh_sb = cpool.tile([F, U], bf16)
            nc.sync.dma_start(wh_sb, Wh[:, :])
            uzr_sb = cpool.tile([U, 2 * U], bf16)
            nc.sync.dma_start(uzr_sb, Uzr[:, :])
            uh_sb = cpool.tile([U, U], bf16)
            nc.sync.dma_start(uh_sb, Uh[:, :])
            bzr_sb = cpool.tile([2 * U, 1], f32)
            nc.sync.dma_start(bzr_sb, bzr[:, :])
            b1h_sb = cpool.tile([U, 1], f32)
            nc.sync.dma_start(b1h_sb, b1h[:, :])
            b0h_sb = cpool.tile([U, 1], f32)
            nc.sync.dma_start(b0h_sb, b0h[:, :])

            # Route consts through DVE copies so PE/Act instrs wait on
            # compute semaphores, not raw DMA semaphores (LDW 1-wait limit).
            def dve_copy(src, shape, dt):
                dst = cpool.tile(shape, dt, name=src.tensor.name + "_c")
                nc.vector.tensor_copy(dst, src)
                return dst

            wzr_c = dve_copy(wzr_sb, [F, 2 * U], bf16)
            wh_c = dve_copy(wh_sb, [F, U], bf16)
            uzr_c = dve_copy(uzr_sb, [U, 2 * U], bf16)
            uh_c = dve_copy(uh_sb, [U, U], bf16)
            bzr_c = dve_copy(bzr_sb, [2 * U, 1], f32)
            b1h_c = dve_copy(b1h_sb, [U, 1], f32)
            b0h_c = dve_copy(b0h_sb, [U, 1], f32)

            # ---- recurrent state (ping-pong, bf16) ----
            hA = hpool.tile([U, BC], bf16, name="hA")
            hB = hpool.tile([U, BC], bf16, name="hB")
            nc.vector.memset(hA, 0.0)

            with (
                tc.tile_pool(name="pzr", bufs=2, space="PSUM") as pZR,
                tc.tile_pool(name="pxh", bufs=3, space="PSUM") as pXH,
                tc.tile_pool(name="ph", bufs=3, space="PSUM") as pH,
            ):
                # absorb DVE const-copy sem threshold on PE
                dummy = pH.tile([U, BC], f32, name="rhps")
                nc.tensor.matmul(dummy, uh_c, uh_c, start=True, stop=True)

                pzr_w = {}
                pxh_w = {}

                def bulk_slice(w):
                    c = (w * WS) // TCH
                    lt = w * WS - c * TCH
                    return xs_tiles[c][:, lt:lt + WS, :]

                def emit_bulk_zr(w):
                    # allocates the window's PSUM tiles + emits the zr bulk MM
                    pzr_w[w] = pZR.tile([2 * U, WS * BC], f32, name="pzr")
                    pxh_w[w] = pXH.tile([U, WS * BC], f32, name="pxh")
                    nc.tensor.matmul(pzr_w[w], wzr_c, bulk_slice(w),
                                     start=True, stop=False,
                                     skip_group_check=True)

                def emit_bulk_xh(w):
                    nc.tensor.matmul(pxh_w[w], wh_c, bulk_slice(w),
                                     start=True, stop=True,
                                     skip_group_check=True)

                emit_bulk_zr(0)
                emit_bulk_xh(0)

                hcur = hA
                hnxt = hB
                D_prev = None
                xh_sb = {}
                for t in range(T):
                    w = t // WS
                    sl = slice((t % WS) * BC, (t % WS + 1) * BC)
                    pzr = pzr_w[w]
                    pxh = pxh_w[w]

                    # bulk MMs for window w+1 lead the step's PE stream at
                    # phases 6/7: their pool-WAR gate (prev step's Act reader)
                    # has already cleared, so they fill the PE idle gap at the
                    # end of the previous step instead of colliding with i1
                    if t % WS == 6 and w + 1 < NW:
                        emit_bulk_zr(w + 1)
                    if t % WS == 7 and w + 1 < NW:
                        emit_bulk_xh(w + 1)

                    if t >= 1:
                        # critical: close pzr[t] with the D-delta accumulate
                        nc.tensor.matmul(pzr[:, sl], uzr_c, D_prev,
                                         start=False, stop=True,
                                         skip_group_check=True)
                    S = spool.tile([2 * U, BC], bf16, name="S")
                    if nzzr:
                        nc.scalar.activation(S, pzr[:, sl], AF.Sigmoid,
                                             bias=bzr_c, scale=1.0)
                    else:
                        nc.scalar.activation(S, pzr[:, sl], AF.Sigmoid)
                    if t >= 1:
                        ph = pH.tile([U, BC], f32, name="rhps")
                        nc.tensor.matmul(ph, uh_c, hcur, start=True, stop=True,
                                         skip_group_check=True)
                        if t + 1 < T:
                            # early accumulate of U^T h_t into pzr[t+1]
                            nxt_w = (t + 1) // WS
                            nsl = slice(((t + 1) % WS) * BC,
                                        ((t + 1) % WS + 1) * BC)
                            nc.tensor.matmul(pzr_w[nxt_w][:, nsl], uzr_c, hcur,
                                             start=False, stop=False,
                                             skip_group_check=True)
                    if t % TCH == 0 and t // TCH + 2 < NCHUNK:
                        emit_dma(t // TCH + 2)
                    # stage xh for step t+1 to SBUF (off critical path);
                    # emitted after the bulk pieces so the copy reading
                    # window w+1 sees its writer
                    if t + 1 < T:
                        nw_, nt_ = (t + 1) // WS, (t + 1) % WS
                        xh_n = xhpool.tile([U, BC], bf16, name="xh")
                        nc.scalar.copy(
                            xh_n, pxh_w[nw_][:, nt_ * BC:(nt_ + 1) * BC])
                        xh_sb[t + 1] = xh_n

                    # ---- elementwise chain (DVE) ----
                    if t == 0:
                        C = dpool.tile([U, BC], bf16, name="C")
                        nc.vector.tensor_scalar_max(C, pxh[:, sl], 0.0)
                        D = ddpool.tile([U, BC], bf16, name="D")
                        nc.vector.tensor_mul(D, S[:U, :], C)
                        nc.vector.tensor_add(hnxt, hcur, D)
                    else:
                        A = dpool.tile([U, BC], bf16, name="A")
                        if nzrec:
                            nc.vector.scalar_tensor_tensor(
                                A, ph, b1h_c, S[U:2 * U, :],
                                op0=OP.add, op1=OP.mult)
                        else:
                            nc.vector.tensor_mul(A, S[U:2 * U, :], ph)
                        Bt = dpool.tile([U, BC], bf16, name="Bt")
                        if nzb0h:
                            nc.vector.scalar_tensor_tensor(
                                Bt, A, b0h_c, xh_sb[t],
                                op0=OP.add, op1=OP.add)
                        else:
                            nc.vector.tensor_add(Bt, A, xh_sb[t])
                        C = dpool.tile([U, BC], bf16, name="C")
                        nc.vector.scalar_tensor_tensor(
                            C, Bt, 0.0, hcur, op0=OP.max, op1=OP.subtract)
                        D = ddpool.tile([U, BC], bf16, name="D")
                        nc.vector.tensor_mul(D, S[:U, :], C)
                        nc.vector.tensor_add(hnxt, hcur, D)

                    D_prev = D
                    hcur, hnxt = hnxt, hcur

            nc.sync.dma_start(hout[:, :], hcur)

    nc.finalize()
    return nc


_CACHE = {}


def kernel(**inputs) -> np.ndarray:
    global LAST_RESULTS
    x = np.asarray(inputs["inputs"], dtype=np.float32)
    W = np.asarray(inputs["W"], dtype=np.float32)
    Um = np.asarray(inputs["U"], dtype=np.float32)
    b = np.asarray(inputs["b"], dtype=np.float32)
    W1 = np.asarray(inputs["W1"], dtype=np.float32)
    b1 = np.asarray(inputs["b1"], dtype=np.float32)
    W2 = np.asarray(inputs["W2"], dtype=np.float32)
    b2 = np.asarray(inputs["b2"], dtype=np.float32)

    nzrec = bool(np.any(b[1, 2 * U:]))
    nzb0h = bool(np.any(b[0, 2 * U:]))
    bsum = b[0] + b[1]
    bzr = np.concatenate([-bsum[:U], bsum[U:2 * U]]).reshape(2 * U, 1)
    nzzr = bool(np.any(bzr))
    key = (nzrec, nzb0h, nzzr)
    if key not in _CACHE:
        _CACHE[key] = build_nc(nzrec, nzb0h, nzzr)
    nc = _CACHE[key]

    # negate z-columns so sigmoid(pre_z) directly yields 1-z
    Wmod = W.copy()
    Wmod[:, :U] *= -1.0
    Umod = Um.copy()
    Umod[:, :U] *= -1.0
    common = {
        "Wzr": np.ascontiguousarray(Wmod[:, :2 * U]).astype(BF),
        "Wh": np.ascontiguousarray(Wmod[:, 2 * U:]).astype(BF),
        "Uzr": np.ascontiguousarray(Umod[:, :2 * U]).astype(BF),
        "Uh": np.ascontiguousarray(Umod[:, 2 * U:]).astype(BF),
        "bzr": np.ascontiguousarray(bzr, dtype=np.float32),
        "b1h": np.ascontiguousarray(b[1, 2 * U:].reshape(U, 1)),
        "b0h": np.ascontiguousarray(b[0, 2 * U:].reshape(U, 1)),
    }
    in_maps = []
    for c in range(NCORES):
        xc = np.ascontiguousarray(
            x[c * BC:(c + 1) * BC].transpose(2, 1, 0)).astype(BF)
        in_maps.append(dict(common, xT=xc))
    res = run_bass_kernel_spmd(nc, in_maps, core_ids=list(range(NCORES)),
                               trace=TRACE)
    LAST_RESULTS = res

    # MLP head + softmax on host (fp32)
    h_full = np.concatenate(
        [np.asarray(res.results[c]["hout"]).astype(np.float32).T
         for c in range(NCORES)], axis=0)            # [B, U]
    x1 = np.maximum(h_full @ W1 + b1, 0.0)
    lg = x1 @ W2 + b2
    e = np.exp(lg - lg.max(axis=-1, keepdims=True))
    return (e / e.sum(axis=-1, keepdims=True)).astype(np.float32)


# revision 23
# speedup vs baseline: 1.1985x; 1.1985x over previous
import numpy as np
import ml_dtypes

import concourse.bass as bass
import concourse.bacc as bacc
import concourse.tile as tile
from concourse import mybir
from concourse.bass_utils import run_bass_kernel_spmd

B, T, F, U, NCLS = 512, 512, 128, 64, 10
NCORES = 8
BC = B // NCORES          # 64 batch rows per core
WS = 8                    # timesteps per PSUM window
NW = T // WS              # 64 windows
TCH = 64                  # timesteps per DMA chunk
NCHUNK = T // TCH

f32 = mybir.dt.float32
bf16 = mybir.dt.bfloat16
AF = mybir.ActivationFunctionType
OP = mybir.AluOpType
BF = ml_dtypes.bfloat16

TRACE = False
LAST_RESULTS = None


def build_nc(nzrec: bool, nzb0h: bool, nzzr: bool) -> bass.Bass:
    nc = bacc.Bacc(None, target_bir_lowering=False)

    # x pre-transposed on host: [F, T, BC] bf16.  z-columns of Wzr/Uzr are
    # host-negated so the sigmoid needs no scale vector: S[0:U] = sigmoid(-az)
    # = 1-z, S[U:2U] = sigmoid(ar) = r.
    xT = nc.dram_tensor("xT", [F, T, BC], bf16, kind="ExternalInput")
    Wzr = nc.dram_tensor("Wzr", [F, 2 * U], bf16, kind="ExternalInput")
    Wh = nc.dram_tensor("Wh", [F, U], bf16, kind="ExternalInput")
    Uzr = nc.dram_tensor("Uzr", [U, 2 * U], bf16, kind="ExternalInput")
    Uh = nc.dram_tensor("Uh", [U, U], bf16, kind="ExternalInput")
    bzr = nc.dram_tensor("bzr", [2 * U, 1], f32, kind="ExternalInput")
    b1h = nc.dram_tensor("b1h", [U, 1], f32, kind="ExternalInput")
    b0h = nc.dram_tensor("b0h", [U, 1], f32, kind="ExternalInput")
    hout = nc.dram_tensor("hout", [U, BC], bf16, kind="ExternalOutput")

    with tile.TileContext(nc) as tc:
        with (
            tc.tile_pool(name="const", bufs=1) as cpool,
            tc.tile_pool(name="xchunk", bufs=2) as xpool,
            tc.tile_pool(name="hbuf", bufs=1) as hpool,
            tc.tile_pool(name="spool", bufs=4) as spool,
            tc.tile_pool(name="dpool", bufs=4) as dpool,
            tc.tile_pool(name="ddpool", bufs=2) as ddpool,
            tc.tile_pool(name="xhpool", bufs=3) as xhpool,
        ):
            # ---- start the big x-chunk DMAs first (overlap const setup) ----
            xs_tiles = {}

            def emit_dma(c):
                xsb = xpool.tile([F, TCH, BC], bf16, name="xsb")
                nc.sync.dma_start(xsb, xT[:, c * TCH:(c + 1) * TCH, :])
                xs_tiles[c] = xsb

            emit_dma(0)
            emit_dma(1)

            # ---- constants to SBUF ----
            wzr_sb = cpool.tile([F, 2 * U], bf16)
            nc.sync.dma_start(wzr_sb, Wzr[:, :])
            wh_sb = cpool.tile([F, U], bf16)
            nc.sync.dma_start(wh_sb, Wh[:, :])
            uzr_sb = cpool.tile([U, 2 * U], bf16)
            nc.sync.dma_start(uzr_sb, Uzr[:, :])
            uh_sb = cpool.tile([U, U], bf16)
            nc.sync.dma_start(uh_sb, Uh[:, :])
            bzr_sb = cpool.tile([2 * U, 1], f32)
            nc.sync.dma_start(bzr_sb, bzr[:, :])
            b1h_sb = cpool.tile([U, 1], f32)
            nc.sync.dma_start(b1h_sb, b1h[:, :])
            b0h_sb = cpool.tile([U, 1], f32)
            nc.sync.dma_start(b0h_sb, b0h[:, :])

            # Route consts through DVE copies so PE/Act instrs wait on
            # compute semaphores, not raw DMA semaphores (LDW 1-wait limit).
            def dve_copy(src, shape, dt):
                dst = cpool.tile(shape, dt, name=src.tensor.name + "_c")
                nc.vector.tensor_copy(dst, src)
                return dst

            wzr_c = dve_copy(wzr_sb, [F, 2 * U], bf16)
            wh_c = dve_copy(wh_sb, [F, U], bf16)
            uzr_c = dve_copy(uzr_sb, [U, 2 * U], bf16)
            uh_c = dve_copy(uh_sb, [U, U], bf16)
            bzr_c = dve_copy(bzr_sb, [2 * U, 1], f32)
            b1h_c = dve_copy(b1h_sb, [U, 1], f32)
            b0h_c = dve_copy(b0h_sb, [U, 1], f32)

            # ---- recurrent state (ping-pong, bf16) ----
            hA = hpool.tile([U, BC], bf16, name="hA")
            hB = hpool.tile([U, BC], bf16, name="hB")
            nc.vector.memset(hA, 0.0)

            with (
                tc.tile_pool(name="pzr", bufs=2, space="PSUM") as pZR,
                tc.tile_pool(name="pxh", bufs=2, space="PSUM") as pXH,
                tc.tile_pool(name="ph", bufs=3, space="PSUM") as pH,
            ):
                # absorb DVE const-copy sem threshold on PE
                dummy = pH.tile([U, BC], f32, name="rhps")
                nc.tensor.matmul(dummy, uh_c, uh_c, start=True, stop=True)

                pzr_w = {}
                pxh_w = {}

                def bulk_slice(w):
                    c = (w * WS) // TCH
                    lt = w * WS - c * TCH
                    return xs_tiles[c][:, lt:lt + WS, :]

                def emit_bulk_zr(w):
                    # allocates the window's PSUM tiles + emits the zr bulk MM
                    pzr_w[w] = pZR.tile([2 * U, WS * BC], f32, name="pzr")
                    pxh_w[w] = pXH.tile([U, WS * BC], f32, name="pxh")
                    nc.tensor.matmul(pzr_w[w], wzr_c, bulk_slice(w),
                                     start=True, stop=False,
                                     skip_group_check=True)

                def emit_bulk_xh(w):
                    nc.tensor.matmul(pxh_w[w], wh_c, bulk_slice(w),
                                     start=True, stop=True,
                                     skip_group_check=True)

                emit_bulk_zr(0)
                emit_bulk_xh(0)

                hcur = hA
                hnxt = hB
                D_prev = None
                xh_sb = {}
                for t in range(T):
                    w = t // WS
                    sl = slice((t % WS) * BC, (t % WS + 1) * BC)
                    pzr = pzr_w[w]
                    pxh = pxh_w[w]

                    # bulk MMs for window w+1 lead the step's PE stream at
                    # phases 6/7: their pool-WAR gate (prev step's Act reader)
                    # has already cleared, so they fill the PE idle gap at the
                    # end of the previous step instead of colliding with i1
                    if t % WS == 6 and w + 1 < NW:
                        emit_bulk_zr(w + 1)
                    if t % WS == 7 and w + 1 < NW:
                        emit_bulk_xh(w + 1)

                    if t >= 1:
                        # critical: close pzr[t] with the D-delta accumulate
                        nc.tensor.matmul(pzr[:, sl], uzr_c, D_prev,
                                         start=False, stop=True,
                                         skip_group_check=True)
                    S = spool.tile([2 * U, BC], bf16, name="S")
                    if nzzr:
                        nc.scalar.activation(S, pzr[:, sl], AF.Sigmoid,
                                             bias=bzr_c, scale=1.0)
                    else:
                        nc.scalar.activation(S, pzr[:, sl], AF.Sigmoid)
                    if t >= 1:
                        ph = pH.tile([U, BC], f32, name="rhps")
                        nc.tensor.matmul(ph, uh_c, hcur, start=True, stop=True,
                                         skip_group_check=True)
                        if t + 1 < T:
                            # early accumulate of U^T h_t into pzr[t+1]
                            nxt_w = (t + 1) // WS
                            nsl = slice(((t + 1) % WS) * BC,
                                        ((t + 1) % WS + 1) * BC)
                            nc.tensor.matmul(pzr_w[nxt_w][:, nsl], uzr_c, hcur,
                                             start=False, stop=False,
                                             skip_group_check=True)
                    if t % TCH == 0 and t // TCH + 2 < NCHUNK:
                        emit_dma(t // TCH + 2)
                    # stage xh for step t+1 to SBUF (off critical path);
                    # emitted after the bulk pieces so the copy reading
                    # window w+1 sees its writer
                    if t + 1 < T:
                        nw_, nt_ = (t + 1) // WS, (t + 1) % WS
                        xh_n = xhpool.tile([U, BC], bf16, name="xh")
                        nc.scalar.copy(
                            xh_n, pxh_w[nw_][:, nt_ * BC:(nt_ + 1) * BC])
                        xh_sb[t + 1] = xh_n

                    # ---- elementwise chain (DVE) ----
                    if t == 0:
                        C = dpool.tile([U, BC], bf16, name="C")
                        nc.vector.tensor_scalar_max(C, pxh[:, sl], 0.0)
                        D = ddpool.tile([U, BC], bf16, name="D")
                        nc.vector.tensor_mul(D, S[:U, :], C)
                        nc.vector.tensor_add(hnxt, hcur, D)
                    else:
                        A = dpool.tile([U, BC], bf16, name="A")
                        if nzrec:
                            nc.vector.scalar_tensor_tensor(
                                A, ph, b1h_c, S[U:2 * U, :],
                                op0=OP.add, op1=OP.mult)
                        else:
                            nc.vector.tensor_mul(A, S[U:2 * U, :], ph)
                        Bt = dpool.tile([U, BC], bf16, name="Bt")
                        if nzb0h:
                            nc.vector.scalar_tensor_tensor(
                                Bt, A, b0h_c, xh_sb[t],
                                op0=OP.add, op1=OP.add)
                        else:
                            nc.vector.tensor_add(Bt, A, xh_sb[t])
                        C = dpool.tile([U, BC], bf16, name="C")
                        nc.vector.scalar_tensor_tensor(
                            C, Bt, 0.0, hcur, op0=OP.max, op1=OP.subtract)
                        D = ddpool.tile([U, BC], bf16, name="D")
                        nc.vector.tensor_mul(D, S[:U, :], C)
                        nc.vector.tensor_add(hnxt, hcur, D)

                    D_prev = D
                    hcur, hnxt = hnxt, hcur

            nc.sync.dma_start(hout[:, :], hcur)

    nc.finalize()
    return nc


_CACHE = {}


def kernel(**inputs) -> np.ndarray:
    global LAST_RESULTS
    x = np.asarray(inputs["inputs"], dtype=np.float32)
    W = np.asarray(inputs["W"], dtype=np.float32)
    Um = np.asarray(inputs["U"], dtype=np.float32)
    b = np.asarray(inputs["b"], dtype=np.float32)
    W1 = np.asarray(inputs["W1"], dtype=np.float32)
    b1 = np.asarray(inputs["b1"], dtype=np.float32)
    W2 = np.asarray(inputs["W2"], dtype=np.float32)
    b2 = np.asarray(inputs["b2"], dtype=np.float32)

    nzrec = bool(np.any(b[1, 2 * U:]))
    nzb0h = bool(np.any(b[0, 2 * U:]))
    bsum = b[0] + b[1]
    bzr = np.concatenate([-bsum[:U], bsum[U:2 * U]]).reshape(2 * U, 1)
    nzzr = bool(np.any(bzr))
    key = (nzrec, nzb0h, nzzr)
    if key not in _CACHE:
        _CACHE[key] = build_nc(nzrec, nzb0h, nzzr)
    nc = _CACHE[key]

    # negate z-columns so sigmoid(pre_z) directly yields 1-z
    Wmod = W.copy()
    Wmod[:, :U] *= -1.0
    Umod = Um.copy()
    Umod[:, :U] *= -1.0
    common = {
        "Wzr": np.ascontiguousarray(Wmod[:, :2 * U]).astype(BF),
        "Wh": np.ascontiguousarray(Wmod[:, 2 * U:]).astype(BF),
        "Uzr": np.ascontiguousarray(Umod[:, :2 * U]).astype(BF),
        "Uh": np.ascontiguousarray(Umod[:, 2 * U:]).astype(BF),
        "bzr": np.ascontiguousarray(bzr, dtype=np.float32),
        "b1h": np.ascontiguousarray(b[1, 2 * U:].reshape(U, 1)),
        "b0h": np.ascontiguousarray(b[0, 2 * U:].reshape(U, 1)),
    }
    in_maps = []
    for c in range(NCORES):
        xc = np.ascontiguousarray(
            x[c * BC:(c + 1) * BC].transpose(2, 1, 0)).astype(BF)
        in_maps.append(dict(common, xT=xc))
    res = run_bass_kernel_spmd(nc, in_maps, core_ids=list(range(NCORES)),
                               trace=TRACE)
    LAST_RESULTS = res

    # MLP head + softmax on host (fp32)
    h_full = np.concatenate(
        [np.asarray(res.results[c]["hout"]).astype(np.float32).T
         for c in range(NCORES)], axis=0)            # [B, U]
    x1 = np.maximum(h_full @ W1 + b1, 0.0)
    lg = x1 @ W2 + b2
    e = np.exp(lg - lg.max(axis=-1, keepdims=True))
    return (e / e.sum(axis=-1, keepdims=True)).astype(np.float32)
